# revision 5
# baseline (speedup 1.0000x reference)
"""Trainium2 Bass kernel for nn_Attention (dense transformer block), v3.

v3 = v2 + fp8e4 DoubleRow for the q/k projections only: x and w_q/w_k are
also staged in fp8 with feature planes paired (K=256/matmul, 2 MAC/cell/cyc),
halving the q/k projection PE time. The S->P path tolerates fp8 because the
exp argument is divided by 64 (error ablation: +2e-3). fp8 for v/P/out-proj
was measured at +2-3e-2 each (signed/attenuated signals keep the full
per-element quantization error) and is NOT used. Host prescales w_q/w_k by
32 (exact pow2, avoids fp8 subnormals), compensated via exp scale.

Reference computation (per batch b of 2, seq N=2048, dim D=1024, 16 heads x 64):
    q = (x @ w_q) / 64                      # source double-scales by d**-0.5
    k, v = split(x @ w_kv)
    per head: out_h = softmax(causal(q_h k_h^T)) v_h
    y = concat(out) @ w_out + b_out

Sharding (8 cores): core c -> batch b = c//4, head group g = c%4 (heads 4g..4g+3).
Each core computes its 4 heads end-to-end plus its partial output projection
(rows 256g..256g+256 of w_out); the host sums the 4 partials per batch and adds
b_out.

v2 design (vs v1):
  - all inputs bf16 (host converts); all matmuls bf16 with fp32 PSUM accum.
  - AV lhsT per head is [v_h | ones64] or [ones64 | v_h] (128 cols), so the
    softmax denominator comes out REPLICATED on 64 PSUM partitions; the
    normalization is two DVE ops (reciprocal + multiply) per (pr,c,h2) with
    no PE broadcast, no SBUF->SBUF DMA hop (DVE 64-wide writes may target
    either partition half).
  - engine assignment: ACT does exp only; DVE does PSUM->SBUF copies +
    normalize; GPSIMD (Pool) does the causal triangle masks + ones memset.
  - emission order interleaves projection / output-projection matmuls into
    the attention j-tile stream as background quanta so the PE never idles
    behind the ACT exp stream.
"""

import numpy as np

import concourse.bass as bass
import concourse.mybir as mybir
import concourse.tile as tile
from concourse import bacc
from concourse.bass_utils import run_bass_kernel_spmd

FP = mybir.dt.float32
BF = mybir.dt.bfloat16
F8 = mybir.dt.float8e4
DR = mybir.MatmulPerfMode.DoubleRow
EXP = mybir.ActivationFunctionType.Exp
WS = 32.0  # host prescale on w_q/w_k (exact power of 2)

B = 2
N = 2048  # sequence length
D = 1024  # model dim
NH = 4  # heads per core
DH = 64  # head dim
G = NH * DH  # 256 = per-core projection width
P = 128
DKT = D // P  # 8 feature k-tiles
KT = N // P  # 16 sequence k-tiles
QC = N // 512  # 4 q chunks of 512
NCORES = 8


def build_bass(repeat=1, hw_loop=0, upto="full", loads_once=False):
    nc = bacc.Bacc("TRN2", target_bir_lowering=False, debug=False, num_devices=NCORES)

    xT = nc.dram_tensor("xT", [D, N], BF, kind="ExternalInput").ap()
    xT8 = nc.dram_tensor("xT8", [D, N], F8, kind="ExternalInput").ap()
    wq = nc.dram_tensor("wq", [D, G], F8, kind="ExternalInput").ap()
    wk = nc.dram_tensor("wk", [D, G], F8, kind="ExternalInput").ap()
    wv = nc.dram_tensor("wv", [D, G], BF, kind="ExternalInput").ap()  # cols h0,h2,h1,h3
    wo = nc.dram_tensor("wo", [G, D], BF, kind="ExternalInput").ap()
    tri = nc.dram_tensor("tri", [P, P], BF, kind="ExternalInput").ap()
    y = nc.dram_tensor("y", [N, D], FP, kind="ExternalOutput").ap()

    with tile.TileContext(nc) as tc:
        with (
            tc.tile_pool(name="const", bufs=1) as const,
            tc.tile_pool(name="ptp", bufs=4) as ptp,
            tc.tile_pool(name="ysb", bufs=2) as ysbp,
            tc.tile_pool(name="rsp", bufs=2) as rsp,
            tc.tile_pool(name="psum", bufs=2, space="PSUM") as psum,
        ):
            # fp8 paired layouts [P, t, ko, ...]: feature plane = 2t+ko
            wq_sb = const.tile([P, DKT // 2, 2, G], F8)
            wk_sb = const.tile([P, DKT // 2, 2, G], F8)
            xT8_sb = const.tile([P, DKT // 2, 2, N], F8)
            wv_sb = const.tile([P, DKT, G], BF)
            wo_sb = const.tile([P, 2, D], BF)
            tri_sb = const.tile([P, P], BF)
            xT_sb = const.tile([P, DKT, N], BF)
            qT_sb = const.tile([P, 2, N], BF)
            kT_sb = const.tile([P, 2, N], BF)
            # v planes per (jtile, pr): [v_{2pr} | ones | ones | v_{2pr+1}]
            v_sb = const.tile([P, KT, 2, 256], BF)
            oT_sb = const.tile([P, 2, N], BF)

            xTr = xT.rearrange("(o p) m -> p o m", p=P)
            xT8r = xT8.rearrange("(t k p) n -> p t k n", p=P, k=2)
            wqr = wq.rearrange("(t k p) m -> p t k m", p=P, k=2)
            wkr = wk.rearrange("(t k p) m -> p t k m", p=P, k=2)

            def load_qk_w():
                # per plane-pair, interleaved: the first q/k projection matmuls
                # only need plane t=0, so they start ~3us earlier
                for t in range(DKT // 2):
                    nc.scalar.dma_start(wq_sb[:, t], wqr[:, t])
                    nc.scalar.dma_start(wk_sb[:, t], wkr[:, t])

            def load_rest_w():
                nc.scalar.dma_start(wv_sb, wv.rearrange("(o p) m -> p o m", p=P))
                nc.scalar.dma_start(tri_sb, tri)
                nc.scalar.dma_start(wo_sb, wo.rearrange("(o p) m -> p o m", p=P))

            def load_x8(ch):  # fp8 x chunk (feeds q/k projections)
                nc.sync.dma_start(
                    xT8_sb[:, :, :, 512 * ch : 512 * (ch + 1)],
                    xT8r[:, :, :, 512 * ch : 512 * (ch + 1)],
                )

            def load_x(ch):  # bf16 x chunk (feeds v projection)
                nc.sync.dma_start(
                    xT_sb[:, :, 512 * ch : 512 * (ch + 1)],
                    xTr[:, :, 512 * ch : 512 * (ch + 1)],
                )

            def ones_memset():
                nc.gpsimd.memset(v_sb[:, :, :, 64:192], 1.0)

            # ---- background quanta -------------------------------------
            def pq(w_sb, dst, pl, half):
                # dst[:, pl, 512*half:+512] = (w plane pl)^T @ xT cols
                def mms():
                    ps = psum.tile([P, 512], FP, tag="rb", name=f"ps_p{pl}h{half}")
                    lo = 512 * half
                    for t in range(DKT // 2):
                        yield nc.tensor.matmul(
                            ps,
                            w_sb[:, t, :, P * pl : P * (pl + 1)],
                            xT8_sb[:, t, :, lo : lo + 512],
                            start=(t == 0),
                            stop=(t == DKT // 2 - 1),
                            perf_mode=DR,
                        )
                    nc.vector.tensor_copy(dst[:, pl, lo : lo + 512], ps)

                return mms

            def pv(jt):
                # v rows 128*jt..+128, all 4 heads; wv host col order h0,h2,h1,h3
                def mms():
                    ps = psum.tile([P, G], FP, tag="rb", name="ps_v")
                    for j in range(DKT):
                        yield nc.tensor.matmul(
                            ps,
                            xT_sb[:, j, P * jt : P * (jt + 1)],
                            wv_sb[:, j, :],
                            start=(j == 0),
                            stop=(j == DKT - 1),
                        )
                    psv = ps.rearrange("p (a d) -> p a d", a=4)
                    nc.vector.tensor_copy(
                        v_sb[:, jt, :, 0:64], psv[:, 0:2, :]
                    )
                    nc.vector.tensor_copy(
                        v_sb[:, jt, :, 192:256], psv[:, 2:4, :]
                    )

                return mms

            def op(i, on_act=False):
                # y rows 128*i..+128
                def mms():
                    ysb = ysbp.tile([P, D], FP, name="ysb")
                    for n2 in range(2):
                        ps = psum.tile([P, 512], FP, tag="rb", name="ps_y")
                        for pr in range(2):
                            yield nc.tensor.matmul(
                                ps,
                                oT_sb[:, pr, P * i : P * (i + 1)],
                                wo_sb[:, pr, 512 * n2 : 512 * (n2 + 1)],
                                start=(pr == 0),
                                stop=(pr == 1),
                            )
                        dst = ysb[:, 512 * n2 : 512 * (n2 + 1)]
                        if on_act:  # tail: ACT is idle once attention is done
                            nc.scalar.copy(dst, ps)
                        else:
                            nc.vector.tensor_copy(dst, ps)
                        nc.sync.dma_start(
                            y[P * i : P * (i + 1), 512 * n2 : 512 * (n2 + 1)],
                            ysb[:, 512 * n2 : 512 * (n2 + 1)],
                        )

                return mms

            class BG:
                """Round-robin drain of background matmul generators."""

                def __init__(self):
                    self.gens = []

                def add(self, *items):
                    self.gens.extend(g() for g in items)

                def drain(self, n_mms):
                    done = 0
                    while self.gens and done < n_mms:
                        g = self.gens[0]
                        try:
                            next(g)
                            done += 1
                        except StopIteration:
                            self.gens.pop(0)
                    return done

                def drain_all(self):
                    self.drain(1 << 30)

            # ---- attention ---------------------------------------------
            def attn_chunk(bg, pr, c, per_tile, split_norm=False):
                # heads (2*pr, 2*pr+1); q columns 512*c..+512
                outs = [
                    psum.tile([P, 512], FP, tag="o", name=f"o{h2}") for h2 in range(2)
                ]
                rsb = rsp.tile([P, 512], FP, name="rsb")
                last = 4 * c + 3
                pending = None

                def emit_av(item):
                    j, off, PT = item
                    for h2 in range(2):
                        nc.tensor.matmul(
                            outs[h2][:, off:512],
                            v_sb[:, j, pr, 128 * h2 : 128 * (h2 + 1)],
                            PT[:, 512 * h2 + off : 512 * (h2 + 1)],
                            start=(j == 0),
                            stop=(j == last),
                        )

                for j in range(4 * c + 4):
                    off = P * (j - 4 * c) if j >= 4 * c else 0
                    S = psum.tile([P, 1024], FP, tag="s", name="S")
                    for h2 in range(2):
                        base = 64 * h2
                        nc.tensor.matmul(
                            S[:, 512 * h2 + off : 512 * (h2 + 1)],
                            kT_sb[base : base + 64, pr, P * j : P * (j + 1)],
                            qT_sb[base : base + 64, pr, 512 * c + off : 512 * (c + 1)],
                        )
                    PT = ptp.tile([P, 1024], BF, tag="pt", name="PT")
                    if off == 0:
                        nc.scalar.activation(PT, S, EXP, scale=1.0 / (DH * WS * WS))
                    else:
                        sv = S.rearrange("p (h q) -> p h q", h=2)[:, :, off:512]
                        pv_ = PT.rearrange("p (h q) -> p h q", h=2)[:, :, off:512]
                        nc.scalar.activation(pv_, sv, EXP, scale=1.0 / (DH * WS * WS))
                    if j >= 4 * c:  # diagonal tile: triangle mask on GPSIMD
                        for h2 in range(2):
                            sl = slice(512 * h2 + off, 512 * h2 + off + P)
                            nc.gpsimd.tensor_mul(PT[:, sl], PT[:, sl], tri_sb)
                    if pending is not None:
                        emit_av(pending)
                    pending = (j, off, PT)
                    bg.drain(per_tile)
                emit_av(pending)
                # normalize: head h2 dims are at partitions 64*h2..+64 of outs
                # (v plane column order [v|1|1|v]); denominator is replicated on
                # the OTHER 64 partitions. recip crosses partition halves
                # (legal for 64-wide DVE writes); multiply is half-aligned.
                cols = slice(512 * c, 512 * (c + 1))
                for h2 in range(2):
                    d = slice(64 * h2, 64 * h2 + 64)  # dims rows
                    e = slice(64 - 64 * h2, 128 - 64 * h2)  # denom rows
                    nc.vector.reciprocal(rsb[d, :], outs[h2][e, :])
                    nc.vector.tensor_mul(oT_sb[d, pr, cols], outs[h2][d, :], rsb[d, :])

            # ---- program ----------------------------------------------
            def emit_program(skip_loads=False):
                if not skip_loads:
                    load_qk_w()
                    load_x8(0)
                    load_x8(1)
                    load_x(0)
                    load_rest_w()
                    load_x8(2)
                    load_x8(3)
                    load_x(1)
                    load_x(2)
                    load_x(3)
                ones_memset()

                bg = BG()
                # prologue foreground: q/k half 0 for both planes + v tile 0
                for g in (
                    pq(wq_sb, qT_sb, 0, 0),
                    pq(wk_sb, kT_sb, 0, 0),
                    pq(wq_sb, qT_sb, 1, 0),
                    pq(wk_sb, kT_sb, 1, 0),
                    pv(0),
                ):
                    bg.add(g)
                bg.drain_all()

                if upto == "proj":
                    for g in [pv(jt) for jt in range(1, KT)] + [
                        pq(m, d, pl, h)
                        for (m, d) in ((wq_sb, qT_sb), (wk_sb, kT_sb))
                        for pl in range(2)
                        for h in range(1, 4)
                    ]:
                        bg.add(g)
                    bg.drain_all()
                    return

                plan = [
                    # (pr, c, bg items made available before this chunk)
                    (0, 0, [pv(1), pv(2), pv(3)]),
                    (1, 0, [pq(wq_sb, qT_sb, 0, 1), pq(wk_sb, kT_sb, 0, 1), pv(4), pv(5)]),
                    (0, 1, [pv(6), pv(7), pq(wq_sb, qT_sb, 1, 1), pq(wk_sb, kT_sb, 1, 1)]),
                    (1, 1, [pq(wq_sb, qT_sb, 0, 2), pq(wk_sb, kT_sb, 0, 2), pv(8), pv(9), op(0)]),
                    (0, 2, [pv(10), pv(11), pq(wq_sb, qT_sb, 1, 2), pq(wk_sb, kT_sb, 1, 2), op(1), op(2), op(3)]),
                    (1, 2, [pv(12), pv(13), pq(wq_sb, qT_sb, 0, 3), pq(wk_sb, kT_sb, 0, 3), op(4), op(5)]),
                    (0, 3, [pv(14), pv(15), pq(wq_sb, qT_sb, 1, 3), pq(wk_sb, kT_sb, 1, 3), op(6), op(7)]),
                    (1, 3, [op(8), op(9), op(10), op(11)]),
                ]
                for idx, (pr, c, items) in enumerate(plan):
                    bg.add(*items)
                    ntiles = 4 * c + 4
                    per_tile = max(1, (len(items) * 8) // ntiles + 1)
                    if upto == "full":
                        attn_chunk(bg, pr, c, per_tile, split_norm=(idx == len(plan) - 1))
                    else:
                        bg.drain_all()
                bg.drain_all()
                if upto == "full":
                    for i in range(12, 16):
                        bg.add(op(i, on_act=True))
                    bg.drain_all()

            if hw_loop:
                if loads_once:
                    load_qk_w()
                    for ch in range(4):
                        load_x8(ch)
                        load_x(ch)
                    load_rest_w()
                with tc.For_i(0, hw_loop, 1) as _i:
                    for _rep in range(repeat):
                        emit_program(skip_loads=loads_once)
            else:
                for _rep in range(repeat):
                    emit_program()

    nc.compile()
    return nc


_NC = None


def _get_nc():
    global _NC
    if _NC is None:
        _NC = build_bass()
    return _NC


def _bf16(a):
    import ml_dtypes

    return np.asarray(a, dtype=np.float32).astype(ml_dtypes.bfloat16)


def _f8(a):
    import ml_dtypes

    return np.asarray(a, dtype=np.float32).astype(ml_dtypes.float8_e4m3fn)


def make_in_maps(x, w_q, w_kv, w_out):
    tri = np.triu(np.ones((P, P), dtype=np.float32))
    xTs = [np.ascontiguousarray(np.asarray(x[b], dtype=np.float32).T) for b in range(B)]
    w_q = np.asarray(w_q, dtype=np.float32)
    w_kv = np.asarray(w_kv, dtype=np.float32)
    w_out = np.asarray(w_out, dtype=np.float32)
    head_perm = np.r_[0:64, 128:192, 64:128, 192:256]  # h0,h2,h1,h3
    in_maps = []
    for c in range(NCORES):
        b, g = divmod(c, NCORES // B)
        wv_cols = w_kv[:, D + G * g : D + G * (g + 1)][:, head_perm]
        in_maps.append(
            {
                "xT": _bf16(xTs[b]),
                "xT8": _f8(xTs[b]),
                "wq": _f8(w_q[:, G * g : G * (g + 1)] * WS),
                "wk": _f8(w_kv[:, G * g : G * (g + 1)] * WS),
                "wv": _bf16(wv_cols),
                "wo": _bf16(w_out[G * g : G * (g + 1), :]),
                "tri": _bf16(tri),
            }
        )
    return in_maps


def combine_outputs(results, b_out):
    b_out = np.asarray(b_out, dtype=np.float32)
    y = np.zeros((B, N, D), dtype=np.float32)
    for c in range(NCORES):
        y[c // (NCORES // B)] += results[c]["y"]
    y += b_out
    return y


def kernel(x, w_q, w_kv, w_out, b_out):
    nc = _get_nc()
    in_maps = make_in_maps(x, w_q, w_kv, w_out)
    res = run_bass_kernel_spmd(nc, in_maps, core_ids=list(range(NCORES)))
    return combine_outputs(res.results, b_out)


# revision 6
# speedup vs baseline: 2.1193x; 2.1193x over previous
"""Trainium2 Bass kernel for nn_Attention (dense transformer block), v3.

v3 = v2 + fp8e4 DoubleRow for the q/k projections only: x and w_q/w_k are
also staged in fp8 with feature planes paired (K=256/matmul, 2 MAC/cell/cyc),
halving the q/k projection PE time. The S->P path tolerates fp8 because the
exp argument is divided by 64 (error ablation: +2e-3). fp8 for v/P/out-proj
was measured at +2-3e-2 each (signed/attenuated signals keep the full
per-element quantization error) and is NOT used. Host prescales w_q/w_k by
32 (exact pow2, avoids fp8 subnormals), compensated via exp scale.

Reference computation (per batch b of 2, seq N=2048, dim D=1024, 16 heads x 64):
    q = (x @ w_q) / 64                      # source double-scales by d**-0.5
    k, v = split(x @ w_kv)
    per head: out_h = softmax(causal(q_h k_h^T)) v_h
    y = concat(out) @ w_out + b_out

Sharding (8 cores): core c -> batch b = c//4, head group g = c%4 (heads 4g..4g+3).
Each core computes its 4 heads end-to-end plus its partial output projection
(rows 256g..256g+256 of w_out); the host sums the 4 partials per batch and adds
b_out.

v2 design (vs v1):
  - all inputs bf16 (host converts); all matmuls bf16 with fp32 PSUM accum.
  - AV lhsT per head is [v_h | ones64] or [ones64 | v_h] (128 cols), so the
    softmax denominator comes out REPLICATED on 64 PSUM partitions; the
    normalization is two DVE ops (reciprocal + multiply) per (pr,c,h2) with
    no PE broadcast, no SBUF->SBUF DMA hop (DVE 64-wide writes may target
    either partition half).
  - engine assignment: ACT does exp only; DVE does PSUM->SBUF copies +
    normalize; GPSIMD (Pool) does the causal triangle masks + ones memset.
  - emission order interleaves projection / output-projection matmuls into
    the attention j-tile stream as background quanta so the PE never idles
    behind the ACT exp stream.
"""

import numpy as np

import concourse.bass as bass
import concourse.mybir as mybir
import concourse.tile as tile
from concourse import bacc
from concourse.bass_utils import run_bass_kernel_spmd

FP = mybir.dt.float32
BF = mybir.dt.bfloat16
F8 = mybir.dt.float8e4
DR = mybir.MatmulPerfMode.DoubleRow
EXP = mybir.ActivationFunctionType.Exp
WS = 32.0  # host prescale on w_q/w_k (exact power of 2)

B = 2
N = 2048  # sequence length
D = 1024  # model dim
NH = 4  # heads per core
DH = 64  # head dim
G = NH * DH  # 256 = per-core projection width
P = 128
DKT = D // P  # 8 feature k-tiles
KT = N // P  # 16 sequence k-tiles
QC = N // 512  # 4 q chunks of 512
NCORES = 8


def build_bass(repeat=1, hw_loop=0, upto="full", loads_once=False):
    nc = bacc.Bacc("TRN2", target_bir_lowering=False, debug=False, num_devices=NCORES)

    xT = nc.dram_tensor("xT", [D, N], BF, kind="ExternalInput").ap()
    xT8 = nc.dram_tensor("xT8", [D, N], F8, kind="ExternalInput").ap()
    wq = nc.dram_tensor("wq", [D, G], F8, kind="ExternalInput").ap()
    wk = nc.dram_tensor("wk", [D, G], F8, kind="ExternalInput").ap()
    wv = nc.dram_tensor("wv", [D, G], BF, kind="ExternalInput").ap()  # cols h0,h2,h1,h3
    wo = nc.dram_tensor("wo", [G, D], BF, kind="ExternalInput").ap()
    tri = nc.dram_tensor("tri", [P, P], BF, kind="ExternalInput").ap()
    y = nc.dram_tensor("y", [N, D], FP, kind="ExternalOutput").ap()

    with tile.TileContext(nc) as tc:
        with (
            tc.tile_pool(name="const", bufs=1) as const,
            tc.tile_pool(name="ptp", bufs=4) as ptp,
            tc.tile_pool(name="ysb", bufs=2) as ysbp,
            tc.tile_pool(name="rsp", bufs=2) as rsp,
            tc.tile_pool(name="psum", bufs=2, space="PSUM") as psum,
        ):
            # fp8 paired layouts [P, t, ko, ...]: feature plane = 2t+ko
            wq_sb = const.tile([P, DKT // 2, 2, G], F8)
            wk_sb = const.tile([P, DKT // 2, 2, G], F8)
            xT8_sb = const.tile([P, DKT // 2, 2, N], F8)
            wv_sb = const.tile([P, DKT, G], BF)
            wo_sb = const.tile([P, 2, D], BF)
            tri_sb = const.tile([P, P], BF)
            xT_sb = const.tile([P, DKT, N], BF)
            # q/k in fp8 DoubleRow layout: partitions = 2 heads x 32 ki,
            # free = (ko, pr, col); head-dim d = 32*ko + ki (host orders w_q/w_k
            # columns per plane as [hA d0-31 | hB d0-31 | hA d32-63 | hB d32-63])
            qT_sb = const.tile([64, 2, 2, N], F8)
            kT_sb = const.tile([64, 2, 2, N], F8)
            # v planes per (jtile, pr): [v_{2pr} | ones | ones | v_{2pr+1}]
            v_sb = const.tile([P, KT, 2, 256], BF)
            oT_sb = const.tile([P, 2, N], BF)

            xTr = xT.rearrange("(o p) m -> p o m", p=P)
            xT8r = xT8.rearrange("(t k p) n -> p t k n", p=P, k=2)
            wqr = wq.rearrange("(t k p) m -> p t k m", p=P, k=2)
            wkr = wk.rearrange("(t k p) m -> p t k m", p=P, k=2)

            def load_qk_w():
                # ACT queue: these all complete before the first exp, so they
                # cost no exp-stream time; plane t=0 first so the first q/k
                # projection matmuls start early
                for t in range(DKT // 2):
                    nc.scalar.dma_start(wq_sb[:, t], wqr[:, t])
                    nc.scalar.dma_start(wk_sb[:, t], wkr[:, t])

            def load_rest_w():
                nc.scalar.dma_start(tri_sb, tri)
                nc.scalar.dma_start(wv_sb, wv.rearrange("(o p) m -> p o m", p=P))
                nc.scalar.dma_start(wo_sb, wo.rearrange("(o p) m -> p o m", p=P))

            def load_x8(ch):  # fp8 x chunk (feeds q/k projections)
                nc.sync.dma_start(
                    xT8_sb[:, :, :, 512 * ch : 512 * (ch + 1)],
                    xT8r[:, :, :, 512 * ch : 512 * (ch + 1)],
                )

            def load_x(ch):  # bf16 x chunk (feeds v projection)
                nc.sync.dma_start(
                    xT_sb[:, :, 512 * ch : 512 * (ch + 1)],
                    xTr[:, :, 512 * ch : 512 * (ch + 1)],
                )

            def ones_memset():
                nc.gpsimd.memset(v_sb[:, :, :, 64:192], 1.0)

            # ---- background quanta -------------------------------------
            def pq(w_sb, dst, pl, half):
                # dst[:, pl, 512*half:+512] = (w plane pl)^T @ xT cols
                def mms():
                    ps = psum.tile([P, 512], FP, tag="rb", name=f"ps_p{pl}h{half}")
                    lo = 512 * half
                    for t in range(DKT // 2):
                        yield nc.tensor.matmul(
                            ps,
                            w_sb[:, t, :, P * pl : P * (pl + 1)],
                            xT8_sb[:, t, :, lo : lo + 512],
                            start=(t == 0),
                            stop=(t == DKT // 2 - 1),
                            perf_mode=DR,
                        )
                    nc.vector.tensor_copy(dst[:, 0, pl, lo : lo + 512], ps[0:64, :])
                    nc.vector.tensor_copy(dst[:, 1, pl, lo : lo + 512], ps[64:128, :])

                return mms

            def pv(jt):
                # v rows 128*jt..+128, all 4 heads; wv host col order h0,h2,h1,h3
                def mms():
                    ps = psum.tile([P, G], FP, tag="rb", name="ps_v")
                    for j in range(DKT):
                        yield nc.tensor.matmul(
                            ps,
                            xT_sb[:, j, P * jt : P * (jt + 1)],
                            wv_sb[:, j, :],
                            start=(j == 0),
                            stop=(j == DKT - 1),
                        )
                    psv = ps.rearrange("p (a d) -> p a d", a=4)
                    nc.vector.tensor_copy(
                        v_sb[:, jt, :, 0:64], psv[:, 0:2, :]
                    )
                    nc.vector.tensor_copy(
                        v_sb[:, jt, :, 192:256], psv[:, 2:4, :]
                    )

                return mms

            def op(i, on_act=False):
                # y rows 128*i..+128
                def mms():
                    ysb = ysbp.tile([P, D], FP, name="ysb")
                    for n2 in range(2):
                        ps = psum.tile([P, 512], FP, tag="rb", name="ps_y")
                        for pr in range(2):
                            yield nc.tensor.matmul(
                                ps,
                                oT_sb[:, pr, P * i : P * (i + 1)],
                                wo_sb[:, pr, 512 * n2 : 512 * (n2 + 1)],
                                start=(pr == 0),
                                stop=(pr == 1),
                            )
                        dst = ysb[:, 512 * n2 : 512 * (n2 + 1)]
                        if on_act:  # tail: ACT is idle once attention is done
                            nc.scalar.copy(dst, ps)
                        else:
                            nc.vector.tensor_copy(dst, ps)
                        nc.sync.dma_start(
                            y[P * i : P * (i + 1), 512 * n2 : 512 * (n2 + 1)],
                            ysb[:, 512 * n2 : 512 * (n2 + 1)],
                        )

                return mms

            class BG:
                """Round-robin drain of background matmul generators."""

                def __init__(self):
                    self.gens = []

                def add(self, *items):
                    self.gens.extend(g() for g in items)

                def drain(self, n_mms):
                    done = 0
                    while self.gens and done < n_mms:
                        g = self.gens[0]
                        try:
                            next(g)
                            done += 1
                        except StopIteration:
                            self.gens.pop(0)
                    return done

                def drain_all(self):
                    self.drain(1 << 30)

            # ---- attention ---------------------------------------------
            def attn_chunk(bg, pr, c, per_tile, split_norm=False):
                # heads (2*pr, 2*pr+1); q columns 512*c..+512
                outs = [
                    psum.tile([P, 512], FP, tag="o", name=f"o{h2}") for h2 in range(2)
                ]
                rsb = rsp.tile([P, 512], FP, name="rsb")
                last = 4 * c + 3
                pending = None

                def emit_av(item):
                    j, off, PT = item
                    for h2 in range(2):
                        nc.tensor.matmul(
                            outs[h2][:, off:512],
                            v_sb[:, j, pr, 128 * h2 : 128 * (h2 + 1)],
                            PT[:, 512 * h2 + off : 512 * (h2 + 1)],
                            start=(j == 0),
                            stop=(j == last),
                        )

                for j in range(4 * c + 4):
                    off = P * (j - 4 * c) if j >= 4 * c else 0
                    S = psum.tile([P, 1024], FP, tag="s", name="S")
                    for h2 in range(2):
                        base = 32 * h2
                        nc.tensor.matmul(
                            S[:, 512 * h2 + off : 512 * (h2 + 1)],
                            kT_sb[base : base + 32, :, pr, P * j : P * (j + 1)],
                            qT_sb[base : base + 32, :, pr, 512 * c + off : 512 * (c + 1)],
                            perf_mode=DR,
                        )
                    PT = ptp.tile([P, 1024], BF, tag="pt", name="PT")
                    if off == 0:
                        nc.scalar.activation(PT, S, EXP, scale=1.0 / (DH * WS * WS))
                    else:
                        sv = S.rearrange("p (h q) -> p h q", h=2)[:, :, off:512]
                        pv_ = PT.rearrange("p (h q) -> p h q", h=2)[:, :, off:512]
                        nc.scalar.activation(pv_, sv, EXP, scale=1.0 / (DH * WS * WS))
                    if j >= 4 * c:  # diagonal tile: triangle mask on GPSIMD
                        for h2 in range(2):
                            sl = slice(512 * h2 + off, 512 * h2 + off + P)
                            nc.gpsimd.tensor_mul(PT[:, sl], PT[:, sl], tri_sb)
                    if pending is not None:
                        emit_av(pending)
                    pending = (j, off, PT)
                    bg.drain(per_tile)
                emit_av(pending)
                # normalize: head h2 dims are at partitions 64*h2..+64 of outs
                # (v plane column order [v|1|1|v]); denominator is replicated on
                # the OTHER 64 partitions. recip crosses partition halves
                # (legal for 64-wide DVE writes); multiply is half-aligned.
                cols = slice(512 * c, 512 * (c + 1))
                for h2 in range(2):
                    d = slice(64 * h2, 64 * h2 + 64)  # dims rows
                    e = slice(64 - 64 * h2, 128 - 64 * h2)  # denom rows
                    nc.vector.reciprocal(rsb[d, :], outs[h2][e, :])
                    nc.vector.tensor_mul(oT_sb[d, pr, cols], outs[h2][d, :], rsb[d, :])

            # ---- program ----------------------------------------------
            def emit_program(skip_loads=False):
                if not skip_loads:
                    load_qk_w()
                    load_x8(0)
                    load_x8(1)
                    load_x(0)
                    load_rest_w()
                    load_x8(2)
                    load_x8(3)
                    load_x(1)
                    load_x(2)
                    load_x(3)
                ones_memset()

                bg = BG()
                # prologue foreground: q/k half 0 for both planes + v tile 0
                for g in (
                    pq(wq_sb, qT_sb, 0, 0),
                    pq(wk_sb, kT_sb, 0, 0),
                    pq(wq_sb, qT_sb, 1, 0),
                    pq(wk_sb, kT_sb, 1, 0),
                    pv(0),
                ):
                    bg.add(g)
                bg.drain_all()

                if upto == "proj":
                    for g in [pv(jt) for jt in range(1, KT)] + [
                        pq(m, d, pl, h)
                        for (m, d) in ((wq_sb, qT_sb), (wk_sb, kT_sb))
                        for pl in range(2)
                        for h in range(1, 4)
                    ]:
                        bg.add(g)
                    bg.drain_all()
                    return

                plan = [
                    # (pr, c, bg items made available before this chunk)
                    (0, 0, [pv(1), pv(2), pv(3)]),
                    (1, 0, [pq(wq_sb, qT_sb, 0, 1), pq(wk_sb, kT_sb, 0, 1), pv(4), pv(5)]),
                    (0, 1, [pv(6), pv(7), pq(wq_sb, qT_sb, 1, 1), pq(wk_sb, kT_sb, 1, 1)]),
                    (1, 1, [pq(wq_sb, qT_sb, 0, 2), pq(wk_sb, kT_sb, 0, 2), pv(8), pv(9), op(0)]),
                    (0, 2, [pv(10), pv(11), pq(wq_sb, qT_sb, 1, 2), pq(wk_sb, kT_sb, 1, 2), op(1), op(2), op(3)]),
                    (1, 2, [pv(12), pv(13), pq(wq_sb, qT_sb, 0, 3), pq(wk_sb, kT_sb, 0, 3), op(4), op(5)]),
                    (0, 3, [pv(14), pv(15), pq(wq_sb, qT_sb, 1, 3), pq(wk_sb, kT_sb, 1, 3), op(6), op(7)]),
                    (1, 3, [op(8), op(9), op(10), op(11)]),
                ]
                for idx, (pr, c, items) in enumerate(plan):
                    bg.add(*items)
                    ntiles = 4 * c + 4
                    per_tile = max(1, (len(items) * 8) // ntiles + 1)
                    if upto == "full":
                        attn_chunk(bg, pr, c, per_tile, split_norm=(idx == len(plan) - 1))
                    else:
                        bg.drain_all()
                bg.drain_all()
                if upto == "full":
                    for i in range(12, 16):
                        bg.add(op(i, on_act=True))
                    bg.drain_all()

            if hw_loop:
                if loads_once:
                    load_qk_w()
                    for ch in range(4):
                        load_x8(ch)
                        load_x(ch)
                    load_rest_w()
                with tc.For_i(0, hw_loop, 1) as _i:
                    for _rep in range(repeat):
                        emit_program(skip_loads=loads_once)
            else:
                for _rep in range(repeat):
                    emit_program()

    nc.compile()
    return nc


_NC = None


def _get_nc():
    global _NC
    if _NC is None:
        _NC = build_bass()
    return _NC


def _bf16(a):
    import ml_dtypes

    return np.asarray(a, dtype=np.float32).astype(ml_dtypes.bfloat16)


def _f8(a):
    import ml_dtypes

    return np.asarray(a, dtype=np.float32).astype(ml_dtypes.float8_e4m3fn)


def make_in_maps(x, w_q, w_kv, w_out):
    tri = np.triu(np.ones((P, P), dtype=np.float32))
    xTs = [np.ascontiguousarray(np.asarray(x[b], dtype=np.float32).T) for b in range(B)]
    w_q = np.asarray(w_q, dtype=np.float32)
    w_kv = np.asarray(w_kv, dtype=np.float32)
    w_out = np.asarray(w_out, dtype=np.float32)
    head_perm = np.r_[0:64, 128:192, 64:128, 192:256]  # h0,h2,h1,h3
    # per 128-col plane: [hA d0-31 | hB d0-31 | hA d32-63 | hB d32-63]
    qk_perm = np.concatenate(
        [
            np.r_[b + 0:b + 32, b + 64:b + 96, b + 32:b + 64, b + 96:b + 128]
            for b in (0, 128)
        ]
    )
    in_maps = []
    for c in range(NCORES):
        b, g = divmod(c, NCORES // B)
        wv_cols = w_kv[:, D + G * g : D + G * (g + 1)][:, head_perm]
        in_maps.append(
            {
                "xT": _bf16(xTs[b]),
                "xT8": _f8(xTs[b]),
                "wq": _f8(w_q[:, G * g : G * (g + 1)][:, qk_perm] * WS),
                "wk": _f8(w_kv[:, G * g : G * (g + 1)][:, qk_perm] * WS),
                "wv": _bf16(wv_cols),
                "wo": _bf16(w_out[G * g : G * (g + 1), :]),
                "tri": _bf16(tri),
            }
        )
    return in_maps


def combine_outputs(results, b_out):
    b_out = np.asarray(b_out, dtype=np.float32)
    y = np.zeros((B, N, D), dtype=np.float32)
    for c in range(NCORES):
        y[c // (NCORES // B)] += results[c]["y"]
    y += b_out
    return y


def kernel(x, w_q, w_kv, w_out, b_out):
    nc = _get_nc()
    in_maps = make_in_maps(x, w_q, w_kv, w_out)
    res = run_bass_kernel_spmd(nc, in_maps, core_ids=list(range(NCORES)))
    return combine_outputs(res.results, b_out)


# revision 7
# speedup vs baseline: 2.6313x; 1.2416x over previous
"""Trainium2 Bass kernel for nn_Attention (dense transformer block), v3.

v3 = v2 + fp8e4 DoubleRow for the q/k projections only: x and w_q/w_k are
also staged in fp8 with feature planes paired (K=256/matmul, 2 MAC/cell/cyc),
halving the q/k projection PE time. The S->P path tolerates fp8 because the
exp argument is divided by 64 (error ablation: +2e-3). fp8 for v/P/out-proj
was measured at +2-3e-2 each (signed/attenuated signals keep the full
per-element quantization error) and is NOT used. Host prescales w_q/w_k by
32 (exact pow2, avoids fp8 subnormals), compensated via exp scale.

Reference computation (per batch b of 2, seq N=2048, dim D=1024, 16 heads x 64):
    q = (x @ w_q) / 64                      # source double-scales by d**-0.5
    k, v = split(x @ w_kv)
    per head: out_h = softmax(causal(q_h k_h^T)) v_h
    y = concat(out) @ w_out + b_out

Sharding (8 cores): core c -> batch b = c//4, head group g = c%4 (heads 4g..4g+3).
Each core computes its 4 heads end-to-end plus its partial output projection
(rows 256g..256g+256 of w_out); the host sums the 4 partials per batch and adds
b_out.

v2 design (vs v1):
  - all inputs bf16 (host converts); all matmuls bf16 with fp32 PSUM accum.
  - AV lhsT per head is [v_h | ones64] or [ones64 | v_h] (128 cols), so the
    softmax denominator comes out REPLICATED on 64 PSUM partitions; the
    normalization is two DVE ops (reciprocal + multiply) per (pr,c,h2) with
    no PE broadcast, no SBUF->SBUF DMA hop (DVE 64-wide writes may target
    either partition half).
  - engine assignment: ACT does exp only; DVE does PSUM->SBUF copies +
    normalize; GPSIMD (Pool) does the causal triangle masks + ones memset.
  - emission order interleaves projection / output-projection matmuls into
    the attention j-tile stream as background quanta so the PE never idles
    behind the ACT exp stream.
"""

import numpy as np

import concourse.bass as bass
import concourse.mybir as mybir
import concourse.tile as tile
from concourse import bacc
from concourse.bass_utils import run_bass_kernel_spmd

FP = mybir.dt.float32
BF = mybir.dt.bfloat16
F8 = mybir.dt.float8e4
DR = mybir.MatmulPerfMode.DoubleRow
EXP = mybir.ActivationFunctionType.Exp
WS = 32.0  # host prescale on w_q/w_k (exact power of 2)

B = 2
N = 2048  # sequence length
D = 1024  # model dim
NH = 4  # heads per core
DH = 64  # head dim
G = NH * DH  # 256 = per-core projection width
P = 128
DKT = D // P  # 8 feature k-tiles
KT = N // P  # 16 sequence k-tiles
QC = N // 512  # 4 q chunks of 512
NCORES = 8


def build_bass(repeat=1, hw_loop=0, upto="full", loads_once=False):
    nc = bacc.Bacc("TRN2", target_bir_lowering=False, debug=False, num_devices=NCORES)

    xT = nc.dram_tensor("xT", [D, N], BF, kind="ExternalInput").ap()
    xT8 = nc.dram_tensor("xT8", [D, N], F8, kind="ExternalInput").ap()
    wq = nc.dram_tensor("wq", [D, G], F8, kind="ExternalInput").ap()
    wk = nc.dram_tensor("wk", [D, G], F8, kind="ExternalInput").ap()
    wv = nc.dram_tensor("wv", [D, G], BF, kind="ExternalInput").ap()  # cols h0,h2,h1,h3
    wo = nc.dram_tensor("wo", [G, D], BF, kind="ExternalInput").ap()
    tri = nc.dram_tensor("tri", [P, P], BF, kind="ExternalInput").ap()
    y = nc.dram_tensor("y", [N, D], FP, kind="ExternalOutput").ap()

    with tile.TileContext(nc) as tc:
        with (
            tc.tile_pool(name="const", bufs=1) as const,
            tc.tile_pool(name="ptp", bufs=4) as ptp,
            tc.tile_pool(name="ysb", bufs=2) as ysbp,
            tc.tile_pool(name="rsp", bufs=2) as rsp,
            tc.tile_pool(name="psum", bufs=2, space="PSUM") as psum,
        ):
            # fp8 paired layouts [P, t, ko, ...]: feature plane = 2t+ko
            wq_sb = const.tile([P, DKT // 2, 2, G], F8)
            wk_sb = const.tile([P, DKT // 2, 2, G], F8)
            xT8_sb = const.tile([P, DKT // 2, 2, N], F8)
            wv_sb = const.tile([P, DKT, G], BF)
            wo_sb = const.tile([P, 2, D], BF)
            tri_sb = const.tile([P, P], BF)
            xT_sb = const.tile([P, DKT, N], BF)
            qT_sb = const.tile([P, 2, N], BF)
            kT_sb = const.tile([P, 2, N], BF)
            # v planes per (jtile, pr): [v_{2pr} | ones | ones | v_{2pr+1}]
            v_sb = const.tile([P, KT, 2, 256], BF)
            oT_sb = const.tile([P, 2, N], BF)

            xTr = xT.rearrange("(o p) m -> p o m", p=P)
            xT8r = xT8.rearrange("(t k p) n -> p t k n", p=P, k=2)
            wqr = wq.rearrange("(t k p) m -> p t k m", p=P, k=2)
            wkr = wk.rearrange("(t k p) m -> p t k m", p=P, k=2)

            def load_qk_w():
                # per plane-pair, interleaved: the first q/k projection matmuls
                # only need plane t=0, so they start ~3us earlier
                for t in range(DKT // 2):
                    nc.scalar.dma_start(wq_sb[:, t], wqr[:, t])
                    nc.scalar.dma_start(wk_sb[:, t], wkr[:, t])

            def load_rest_w():
                nc.scalar.dma_start(wv_sb, wv.rearrange("(o p) m -> p o m", p=P))
                nc.scalar.dma_start(tri_sb, tri)
                nc.scalar.dma_start(wo_sb, wo.rearrange("(o p) m -> p o m", p=P))

            def load_x8(ch):  # fp8 x chunk (feeds q/k projections)
                nc.sync.dma_start(
                    xT8_sb[:, :, :, 512 * ch : 512 * (ch + 1)],
                    xT8r[:, :, :, 512 * ch : 512 * (ch + 1)],
                )

            def load_x(ch):  # bf16 x chunk (feeds v projection)
                nc.sync.dma_start(
                    xT_sb[:, :, 512 * ch : 512 * (ch + 1)],
                    xTr[:, :, 512 * ch : 512 * (ch + 1)],
                )

            def ones_memset():
                nc.gpsimd.memset(v_sb[:, :, :, 64:192], 1.0)

            # ---- background quanta -------------------------------------
            def pq(w_sb, dst, pl, half):
                # dst[:, pl, 512*half:+512] = (w plane pl)^T @ xT cols
                def mms():
                    ps = psum.tile([P, 512], FP, tag="rb", name=f"ps_p{pl}h{half}")
                    lo = 512 * half
                    for t in range(DKT // 2):
                        yield nc.tensor.matmul(
                            ps,
                            w_sb[:, t, :, P * pl : P * (pl + 1)],
                            xT8_sb[:, t, :, lo : lo + 512],
                            start=(t == 0),
                            stop=(t == DKT // 2 - 1),
                            perf_mode=DR,
                        )
                    nc.vector.tensor_copy(dst[:, pl, lo : lo + 512], ps)

                return mms

            def pv(jt):
                # v rows 128*jt..+128, all 4 heads; wv host col order h0,h2,h1,h3
                def mms():
                    ps = psum.tile([P, G], FP, tag="rb", name="ps_v")
                    for j in range(DKT):
                        yield nc.tensor.matmul(
                            ps,
                            xT_sb[:, j, P * jt : P * (jt + 1)],
                            wv_sb[:, j, :],
                            start=(j == 0),
                            stop=(j == DKT - 1),
                        )
                    psv = ps.rearrange("p (a d) -> p a d", a=4)
                    nc.vector.tensor_copy(
                        v_sb[:, jt, :, 0:64], psv[:, 0:2, :]
                    )
                    nc.vector.tensor_copy(
                        v_sb[:, jt, :, 192:256], psv[:, 2:4, :]
                    )

                return mms

            def op(i, on_act=False):
                # y rows 128*i..+128
                def mms():
                    ysb = ysbp.tile([P, D], FP, name="ysb")
                    for n2 in range(2):
                        ps = psum.tile([P, 512], FP, tag="rb", name="ps_y")
                        for pr in range(2):
                            yield nc.tensor.matmul(
                                ps,
                                oT_sb[:, pr, P * i : P * (i + 1)],
                                wo_sb[:, pr, 512 * n2 : 512 * (n2 + 1)],
                                start=(pr == 0),
                                stop=(pr == 1),
                            )
                        dst = ysb[:, 512 * n2 : 512 * (n2 + 1)]
                        if on_act:  # tail: ACT is idle once attention is done
                            nc.scalar.copy(dst, ps)
                        else:
                            nc.vector.tensor_copy(dst, ps)
                        nc.sync.dma_start(
                            y[P * i : P * (i + 1), 512 * n2 : 512 * (n2 + 1)],
                            ysb[:, 512 * n2 : 512 * (n2 + 1)],
                        )

                return mms

            class BG:
                """Round-robin drain of background matmul generators."""

                def __init__(self):
                    self.gens = []

                def add(self, *items):
                    self.gens.extend(g() for g in items)

                def drain(self, n_mms):
                    done = 0
                    while self.gens and done < n_mms:
                        g = self.gens[0]
                        try:
                            next(g)
                            done += 1
                        except StopIteration:
                            self.gens.pop(0)
                    return done

                def drain_all(self):
                    self.drain(1 << 30)

            # ---- attention ---------------------------------------------
            def attn_chunk(bg, pr, c, per_tile, split_norm=False):
                # heads (2*pr, 2*pr+1); q columns 512*c..+512
                outs = [
                    psum.tile([P, 512], FP, tag="o", name=f"o{h2}") for h2 in range(2)
                ]
                rsb = rsp.tile([P, 512], FP, name="rsb")
                last = 4 * c + 3
                pending = None

                def emit_av(item):
                    j, off, PT = item
                    for h2 in range(2):
                        nc.tensor.matmul(
                            outs[h2][:, off:512],
                            v_sb[:, j, pr, 128 * h2 : 128 * (h2 + 1)],
                            PT[:, 512 * h2 + off : 512 * (h2 + 1)],
                            start=(j == 0),
                            stop=(j == last),
                        )

                for j in range(4 * c + 4):
                    off = P * (j - 4 * c) if j >= 4 * c else 0
                    S = psum.tile([P, 1024], FP, tag="s", name="S")
                    for h2 in range(2):
                        base = 64 * h2
                        nc.tensor.matmul(
                            S[:, 512 * h2 + off : 512 * (h2 + 1)],
                            kT_sb[base : base + 64, pr, P * j : P * (j + 1)],
                            qT_sb[base : base + 64, pr, 512 * c + off : 512 * (c + 1)],
                        )
                    PT = ptp.tile([P, 1024], BF, tag="pt", name="PT")
                    if off == 0:
                        nc.scalar.activation(PT, S, EXP, scale=1.0 / (DH * WS * WS))
                    else:
                        sv = S.rearrange("p (h q) -> p h q", h=2)[:, :, off:512]
                        pv_ = PT.rearrange("p (h q) -> p h q", h=2)[:, :, off:512]
                        nc.scalar.activation(pv_, sv, EXP, scale=1.0 / (DH * WS * WS))
                    if j >= 4 * c:  # diagonal tile: triangle mask on GPSIMD
                        for h2 in range(2):
                            sl = slice(512 * h2 + off, 512 * h2 + off + P)
                            nc.gpsimd.tensor_mul(PT[:, sl], PT[:, sl], tri_sb)
                    if pending is not None:
                        emit_av(pending)
                    pending = (j, off, PT)
                    bg.drain(per_tile)
                emit_av(pending)
                # normalize: head h2 dims are at partitions 64*h2..+64 of outs
                # (v plane column order [v|1|1|v]); denominator is replicated on
                # the OTHER 64 partitions. recip crosses partition halves
                # (legal for 64-wide DVE writes); multiply is half-aligned.
                cols = slice(512 * c, 512 * (c + 1))
                for h2 in range(2):
                    d = slice(64 * h2, 64 * h2 + 64)  # dims rows
                    e = slice(64 - 64 * h2, 128 - 64 * h2)  # denom rows
                    nc.vector.reciprocal(rsb[d, :], outs[h2][e, :])
                    nc.vector.tensor_mul(oT_sb[d, pr, cols], outs[h2][d, :], rsb[d, :])

            # ---- program ----------------------------------------------
            def emit_program(skip_loads=False):
                if not skip_loads:
                    load_qk_w()
                    load_x8(0)
                    load_x8(1)
                    load_x(0)
                    load_rest_w()
                    load_x8(2)
                    load_x8(3)
                    load_x(1)
                    load_x(2)
                    load_x(3)
                ones_memset()

                bg = BG()
                # prologue foreground: q/k half 0 for both planes + v tile 0
                for g in (
                    pq(wq_sb, qT_sb, 0, 0),
                    pq(wk_sb, kT_sb, 0, 0),
                    pq(wq_sb, qT_sb, 1, 0),
                    pq(wk_sb, kT_sb, 1, 0),
                    pv(0),
                ):
                    bg.add(g)
                bg.drain_all()

                if upto == "proj":
                    for g in [pv(jt) for jt in range(1, KT)] + [
                        pq(m, d, pl, h)
                        for (m, d) in ((wq_sb, qT_sb), (wk_sb, kT_sb))
                        for pl in range(2)
                        for h in range(1, 4)
                    ]:
                        bg.add(g)
                    bg.drain_all()
                    return

                plan = [
                    # (pr, c, bg items made available before this chunk)
                    (0, 0, [pv(1), pv(2), pv(3)]),
                    (1, 0, [pq(wq_sb, qT_sb, 0, 1), pq(wk_sb, kT_sb, 0, 1), pv(4), pv(5)]),
                    (0, 1, [pv(6), pv(7), pq(wq_sb, qT_sb, 1, 1), pq(wk_sb, kT_sb, 1, 1)]),
                    (1, 1, [pq(wq_sb, qT_sb, 0, 2), pq(wk_sb, kT_sb, 0, 2), pv(8), pv(9), op(0)]),
                    (0, 2, [pv(10), pv(11), pq(wq_sb, qT_sb, 1, 2), pq(wk_sb, kT_sb, 1, 2), op(1), op(2), op(3)]),
                    (1, 2, [pv(12), pv(13), pq(wq_sb, qT_sb, 0, 3), pq(wk_sb, kT_sb, 0, 3), op(4), op(5)]),
                    (0, 3, [pv(14), pv(15), pq(wq_sb, qT_sb, 1, 3), pq(wk_sb, kT_sb, 1, 3), op(6), op(7)]),
                    (1, 3, [op(8), op(9), op(10), op(11)]),
                ]
                for idx, (pr, c, items) in enumerate(plan):
                    bg.add(*items)
                    ntiles = 4 * c + 4
                    per_tile = max(1, (len(items) * 8) // ntiles + 1)
                    if upto == "full":
                        attn_chunk(bg, pr, c, per_tile, split_norm=(idx == len(plan) - 1))
                    else:
                        bg.drain_all()
                bg.drain_all()
                if upto == "full":
                    for i in range(12, 16):
                        bg.add(op(i, on_act=True))
                    bg.drain_all()

            if hw_loop:
                if loads_once:
                    load_qk_w()
                    for ch in range(4):
                        load_x8(ch)
                        load_x(ch)
                    load_rest_w()
                with tc.For_i(0, hw_loop, 1) as _i:
                    for _rep in range(repeat):
                        emit_program(skip_loads=loads_once)
            else:
                for _rep in range(repeat):
                    emit_program()

    nc.compile()
    return nc


_NC = None


def _get_nc():
    global _NC
    if _NC is None:
        _NC = build_bass()
    return _NC


def _bf16(a):
    import ml_dtypes

    return np.asarray(a, dtype=np.float32).astype(ml_dtypes.bfloat16)


def _f8(a):
    import ml_dtypes

    return np.asarray(a, dtype=np.float32).astype(ml_dtypes.float8_e4m3fn)


def make_in_maps(x, w_q, w_kv, w_out):
    tri = np.triu(np.ones((P, P), dtype=np.float32))
    xTs = [np.ascontiguousarray(np.asarray(x[b], dtype=np.float32).T) for b in range(B)]
    w_q = np.asarray(w_q, dtype=np.float32)
    w_kv = np.asarray(w_kv, dtype=np.float32)
    w_out = np.asarray(w_out, dtype=np.float32)
    head_perm = np.r_[0:64, 128:192, 64:128, 192:256]  # h0,h2,h1,h3
    in_maps = []
    for c in range(NCORES):
        b, g = divmod(c, NCORES // B)
        wv_cols = w_kv[:, D + G * g : D + G * (g + 1)][:, head_perm]
        in_maps.append(
            {
                "xT": _bf16(xTs[b]),
                "xT8": _f8(xTs[b]),
                "wq": _f8(w_q[:, G * g : G * (g + 1)] * WS),
                "wk": _f8(w_kv[:, G * g : G * (g + 1)] * WS),
                "wv": _bf16(wv_cols),
                "wo": _bf16(w_out[G * g : G * (g + 1), :]),
                "tri": _bf16(tri),
            }
        )
    return in_maps


def combine_outputs(results, b_out):
    b_out = np.asarray(b_out, dtype=np.float32)
    y = np.zeros((B, N, D), dtype=np.float32)
    for c in range(NCORES):
        y[c // (NCORES // B)] += results[c]["y"]
    y += b_out
    return y


def kernel(x, w_q, w_kv, w_out, b_out):
    nc = _get_nc()
    in_maps = make_in_maps(x, w_q, w_kv, w_out)
    res = run_bass_kernel_spmd(nc, in_maps, core_ids=list(range(NCORES)))
    return combine_outputs(res.results, b_out)
